# revision 1
# baseline (speedup 1.0000x reference)
"""Trainium2 Bass kernel for nn_NoduleRecallLoss (segment-reduce recall loss).

Computation (matches the reference):
    fg   = x[:, 1]                    # foreground logits [B,S,S,S]
    yb   = (y > 0) as float           # binary GT
    tp[s]    = sum over voxels with comp_labels==s of fg*yb
    tp_fn[s] = sum over voxels with comp_labels==s of yb
    recall = sum_{s=1..num_components} tp[s]/tp_fn[s]
    loss   = -(recall + 1) / (num_components + 1)

Strategy: data-parallel over 8 NeuronCores (flat voxel sharding). On each
core, labels are radix-decomposed as lab = 16*hi + lo.  The per-segment sums
factor over (lo, hi):
    tp[16h+l]    = sum_v d(lo=l)[v] * (d(hi=h)*fg*yb)[v]
    tp_fn[16h+l] = sum_v d(lo=l)[v] * (d(hi=h)*yb)[v]
DVE perf modes dictate the op mix: plain tensor_scalar comparisons run at
4x, tensor_tensor multiplies at 2x, fused scalar_tensor_tensor only at 1x.
So we build *nested* hi-side masks ge_h = (lab >= 16h) (4x eq-class ops on a
bf16 copy of the labels, no hi-digit extraction needed since
lab >= 16h <=> hi >= h), apply values with TT multiplies, and let the PE
columns accumulate *cumulative* per-hi sums which the host undoes by
differencing.  The lo side needs true digits (mod 16), extracted with uint8
bitwise ops.  The tensor engine contracts each 128-voxel chunk (a column of
the [128, F] tiles):
    psum[16, 18] += lhsT[128, 16]^T @ rhs[128, 18]
with lhsT = [ones, d(lo=1..15)] and
     rhs  = [fgyb, ge_1*fgyb .. ge_8*fgyb, yb, ge_1*yb .. ge_8*yb],
accumulated in PSUM over all chunks.  Host sums the 8 per-core [16, 18]
partials (the per-segment all-reduce), undoes the cumulative-h and
missing-row-0 structure by differencing, and applies the scalar loss.
"""

import sys

sys.path.insert(0, "/opt/trn_rl_repo")

from contextlib import ExitStack

import numpy as np
import ml_dtypes

import concourse.bacc as bacc
import concourse.tile as tile
from concourse import mybir
from concourse.bass_utils import run_bass_kernel_spmd

# Problem geometry (hardcoded per spec).
B = 2
S = 192
NVOX = B * S * S * S  # 14,155,776
NCORES = 8
V8 = NVOX // NCORES  # 1,769,472 voxels per core
P = 128
FT = V8 // P  # 13,824 free columns per core
FTILE = 768  # columns per tile
NT = FT // FTILE  # 18 tiles
NH = 9  # hi digit 0..8
NL = 16  # lo digit 0..15
NCOL = NH * 2  # rhs columns: [fgyb, cum-tp_1..8, yb, cum-fn_1..8]

_BF16 = mybir.dt.bfloat16
_U8 = mybir.dt.uint8
_F32 = mybir.dt.float32
_A = mybir.AluOpType


def _build_program(reps=1):
    nc = bacc.Bacc("TRN2", target_bir_lowering=False)
    fgd = nc.dram_tensor("fgd", [P, NT, FTILE], _BF16, kind="ExternalInput")
    # y and comp_labels as bf16: [P, NT, 2, FTILE] (y, lab)
    ylab = nc.dram_tensor("ylab", [P, NT, 2, FTILE], _BF16, kind="ExternalInput")
    # comp_labels as uint8 for the lo = lab & 15 bitwise extraction
    lab8 = nc.dram_tensor("lab8", [P, NT, FTILE], _U8, kind="ExternalInput")
    out = nc.dram_tensor("out", [NL, NCOL], _F32, kind="ExternalOutput")

    with ExitStack() as ctx:
        tc = ctx.enter_context(tile.TileContext(nc))
        ins = ctx.enter_context(tc.tile_pool(name="ins", bufs=6))
        work = ctx.enter_context(tc.tile_pool(name="work", bufs=2))
        psum = ctx.enter_context(tc.tile_pool(name="psum", bufs=1, space="PSUM"))
        outp = ctx.enter_context(tc.tile_pool(name="outp", bufs=1))

        acc = psum.tile([NL, NCOL], _F32)
        for rep in range(reps):
          for t in range(NT):
            ylab_t = ins.tile([P, 2, FTILE], _BF16, tag="ylab")
            fg_t = ins.tile([P, FTILE], _BF16, tag="fg")
            lab8_t = ins.tile([P, FTILE], _U8, tag="lab8")
            nc.sync.dma_start(out=ylab_t[:], in_=ylab[:, t, :, :])
            nc.sync.dma_start(out=fg_t[:], in_=fgd[:, t, :])
            nc.sync.dma_start(out=lab8_t[:], in_=lab8[:, t, :])
            yb = ylab_t[:, 0, :]
            lab_bf = ylab_t[:, 1, :]

            # lhsT columns: [ones, (lo==1), ..., (lo==15)]
            lhs = work.tile([P, NL, FTILE], _BF16, tag="lhs")
            # rhs columns: [fgyb, ge_1..8 * fgyb, yb->copy, ge_1..8 * yb]
            rhs = work.tile([P, NCOL, FTILE], _BF16, tag="rhs")
            ge = work.tile([P, NH - 1, FTILE], _BF16, tag="ge")
            lo_u8 = work.tile([P, FTILE], _U8, tag="lo8")
            lo_bf = work.tile([P, FTILE], _BF16, tag="lo")
            fgyb = rhs[:, 0, :]

            # lo digit (uint8 bitwise, 1x) + convert to bf16 (1x)
            nc.vector.tensor_scalar(
                out=lo_u8[:], in0=lab8_t[:], scalar1=15, scalar2=None,
                op0=_A.bitwise_and,
            )
            nc.vector.tensor_copy(out=lo_bf[:], in_=lo_u8[:])
            # nested hi masks ge_h = (lab >= 16h)   (eq-class, 4x)
            for h in range(1, NH):
                nc.vector.tensor_scalar(
                    out=ge[:, h - 1, :], in0=lab_bf, scalar1=float(16 * h),
                    scalar2=None, op0=_A.is_ge,
                )
            # fgyb = fg * yb  (TT, 2x); yb copied into rhs col NH (4x copy)
            nc.vector.tensor_tensor(out=fgyb, in0=fg_t[:], in1=yb, op=_A.mult)
            nc.vector.tensor_copy(out=rhs[:, NH, :], in_=yb)
            # masked value columns (TT, 2x) - cumulative in h
            for h in range(1, NH):
                nc.vector.tensor_tensor(
                    out=rhs[:, h, :], in0=ge[:, h - 1, :], in1=fgyb, op=_A.mult
                )
            for h in range(1, NH):
                nc.vector.tensor_tensor(
                    out=rhs[:, NH + h, :], in0=ge[:, h - 1, :], in1=yb, op=_A.mult
                )
            # lo one-hot masks (eq, 4x); row 0 = all-ones (lab >= 0)
            nc.vector.tensor_scalar(
                out=lhs[:, 0, :], in0=lab_bf, scalar1=0.0, scalar2=None,
                op0=_A.is_ge,
            )
            for l in range(1, NL):
                nc.vector.tensor_scalar(
                    out=lhs[:, l, :], in0=lo_bf[:], scalar1=float(l),
                    scalar2=None, op0=_A.is_equal,
                )
            for f in range(FTILE):
                first = t == 0 and f == 0
                last = t == NT - 1 and f == FTILE - 1
                nc.tensor.matmul(
                    acc[:], lhsT=lhs[:, :, f], rhs=rhs[:, :, f],
                    start=first, stop=last,
                )
        res = outp.tile([NL, NCOL], _F32)
        nc.vector.tensor_copy(out=res[:], in_=acc[:])
        nc.sync.dma_start(out=out[:, :], in_=res[:])
    if not nc.is_finalized():
        nc.finalize()
    return nc


_PROGRAM = None


def _get_program():
    global _PROGRAM
    if _PROGRAM is None:
        _PROGRAM = _build_program()
    return _PROGRAM


def make_in_maps(x, y, comp_labels):
    """Host-side sharding + dtype staging (casts/packing only)."""
    bf16 = ml_dtypes.bfloat16
    fg = np.ascontiguousarray(x[:, 1]).reshape(-1).astype(bf16)
    yb = (y.reshape(-1) > 0).astype(bf16)
    labf = comp_labels.reshape(-1).astype(bf16)
    lab8 = comp_labels.reshape(-1).astype(np.uint8)
    in_maps = []
    for c in range(NCORES):
        sl = slice(c * V8, (c + 1) * V8)
        ylab = np.stack(
            [yb[sl].reshape(P, NT, FTILE), labf[sl].reshape(P, NT, FTILE)], axis=2
        )
        in_maps.append(
            {
                "fgd": fg[sl].reshape(P, NT, FTILE),
                "ylab": np.ascontiguousarray(ylab),
                "lab8": lab8[sl].reshape(P, NT, FTILE),
            }
        )
    return in_maps


def combine_partials(outs, num_components):
    """Sum per-core [NL, NCOL] partials -> loss scalar (host all-reduce).

    Raw matrix O[r, c]:
      rows   r=0: all-ones (sum over all lo), r=1..15: d(lo==r)
      cols   c=0..8:  cumulative tp:   sum over hi>=c of tp[lo, hi]
             c=9..17: cumulative tpfn: sum over hi>=c-9 of tpfn[lo, hi]
    """
    O = np.zeros((NL, NCOL), np.float64)
    for o in outs:
        O += o.astype(np.float64)
    # undo cumulative-h by differencing (append zero column)
    Ctp = np.concatenate([O[:, :NH], np.zeros((NL, 1))], axis=1)
    Cfn = np.concatenate([O[:, NH:], np.zeros((NL, 1))], axis=1)
    Tc = Ctp[:, :-1] - Ctp[:, 1:]  # [NL rows(raw), NH]
    Fc = Cfn[:, :-1] - Cfn[:, 1:]
    # undo missing row 0 (row 0 is the sum over all lo)
    T = np.zeros((NL, NH))
    F = np.zeros((NL, NH))
    T[1:, :] = Tc[1:, :]
    T[0, :] = Tc[0, :] - Tc[1:, :].sum(axis=0)
    F[1:, :] = Fc[1:, :]
    F[0, :] = Fc[0, :] - Fc[1:, :].sum(axis=0)
    tp = np.zeros(NL * NH, np.float64)
    tpfn = np.zeros(NL * NH, np.float64)
    for h in range(NH):
        for l in range(NL):
            tp[16 * h + l] = T[l, h]
            tpfn[16 * h + l] = F[l, h]
    n = int(num_components)
    with np.errstate(divide="ignore", invalid="ignore"):
        recall = np.sum(tp[1 : n + 1] / tpfn[1 : n + 1])
    loss = -(recall + 1.0) / (n + 1.0)
    return np.float32(loss)


def kernel(x, y, comp_labels, num_components):
    nc = _get_program()
    in_maps = make_in_maps(np.asarray(x), np.asarray(y), np.asarray(comp_labels))
    res = run_bass_kernel_spmd(nc, in_maps, list(range(NCORES)))
    outs = [res.results[c]["out"] for c in range(NCORES)]
    return combine_partials(outs, np.asarray(num_components))

